# revision 33
# baseline (speedup 1.0000x reference)
"""Trainium2 Bass kernel for the CriticBaseline problem.

reference:
    G = discounted_returns(rewards)            # reverse scan, gamma=0.99
    h = relu(obs @ W1 + b1); h = relu(h @ W2 + b2)
    V = (h @ W3 + b3).reshape(-1)
    return G - V                               # [T]

Strategy (8 NeuronCores, SPMD, no collectives):
  - Data-parallel over T: core c owns timesteps [c*8192, (c+1)*8192).
  - The two 1024x1024 GEMMs run on the PE array in fp8 (e4m3) with
    perf_mode=DoubleRow: the array holds 2 fp8 weights per cell, so one
    matmul contracts 256 inputs (a [128, 2, 128] stationary block) while
    streaming 512 timestep columns -- half the PE cycles of fp32r.
    Weights are host-packed into the [p, g, i, h] pair layout (d =
    256g + 128i + p) and pre-scaled by 32 so sigma~1 lands in the fp8
    normal range; the Relu activations apply scale=1/32 on the way out
    of PSUM. obs is host-cast to fp8 in the same pair layout; h1 is
    written back to SBUF directly in fp8 pair layout by the Relu.
    Timestep tiles are processed in pairs (2 x 512) with both matmuls of
    a (ho, g) weight block issued back to back, so a stationary load can
    serve two 512-column streams.
  - The output error budget tolerates fp8 easily: the returned G - V is
    dominated by G (sigma ~7, computed in fp32) while the fp8-quantized
    V (sigma ~0.7) only contributes ~3-5% of its own magnitude in error.
  - Layer 3 (h2 @ W3) is a per-partition scale + cross-partition reduce:
    DVE accumulates sum_hj h2[hj]*W3[hj] into a [128, 512] bf16 lane
    tile, then the 128-lane sum runs on the otherwise-idle GpSimd engine
    (partition_all_reduce ucode, ~3.8us, fully hidden under the matmul
    stream) and one [1, 512] V row per tile is DMA'd out. The last pair
    instead uses a ones-vector matmul on the PE (216ns) so the program
    doesn't end on two serial GpSimd ucode calls. G (from the scan) and
    V ship as separate outputs; the host fuses out = G - b3 - V while
    un-permuting G's [t%128, t/128] tile layout.
  - The discounted-return scan is computed per-core as a *banded* matmul:
    gamma^k decays below 1.2e-9 by k=2048, so G[i] only needs the next
    2048 rewards. Each core gets its reward slice plus a 2048 overlap
    (zero-padded at the global end) and computes G with 17
    host-precomputed [128,128] coefficient matrices in bf16 (~0.03%
    error on G) accumulated in fp32 PSUM.
  - Startup: a packed consts tensor (w3|b1|b2|b3) lands first so the
    first Relu's bias is never queued behind bulk traffic; pair-0 obs
    and W1 arrive in contraction-group chunks so the PE starts ~10us in;
    ~3.5us of dummy bf16 transposes during the DMA-wait head release the
    PE HAM clock throttle (cold 1.2 GHz -> 2.4 GHz) before the real
    stream; the scalar engine's activation table is pre-loaded the same
    way.
"""

import numpy as np

GAMMA = 0.99
T, D, H = 65536, 1024, 1024
N_CORES = 8
TC = T // N_CORES  # 8192 timesteps per core
TT = 512           # moving-dim tile per matmul stream
NP = 8             # pairs of t-tiles per core (2*512 timesteps each)
NG = 4             # contraction groups of 256 (DoubleRow pairs of 128)
NB = TC // 128     # 64 blocks of 128 timesteps
WIN = 2048         # scan window: gamma^2048 ~ 1.1e-9
NJ = WIN // 128    # 16 -> coefficient matrices j = 0..16
RCOLS = NB + NJ    # 80 columns of packed rewards per core
WSCALE = 32.0      # fp8 weight pre-scale (W1/W2 have sigma 1/32)

_cache = {}


def _scan_mats() -> np.ndarray:
    """Mj[k, p] = gamma^(128j + k - p) on the band 0 <= 128j+k-p < WIN."""
    k = np.arange(128)[:, None]
    p = np.arange(128)[None, :]
    mats = []
    for j in range(NJ + 1):
        e = 128 * j + k - p
        m = np.where((e >= 0) & (e < WIN), np.power(GAMMA, e, dtype=np.float64), 0.0)
        mats.append(m.astype(np.float32))
    return np.ascontiguousarray(np.stack(mats))


def _build():
    """Build + schedule the single-core SPMD Bass program (cached)."""
    if "nc" in _cache:
        return _cache["nc"]

    from contextlib import ExitStack

    import concourse.mybir as mybir
    import concourse.tile as tile
    from concourse import bacc
    from concourse import bass_isa
    from concourse.alu_op_type import AluOpType
    from concourse.masks import make_identity

    f32 = mybir.dt.float32
    bf16 = mybir.dt.bfloat16
    f8 = mybir.dt.float8e4
    DR = mybir.MatmulPerfMode.DoubleRow
    Relu = mybir.ActivationFunctionType.Relu
    Copy = mybir.ActivationFunctionType.Copy
    AX = mybir.AxisListType.X

    nc = bacc.Bacc("TRN2", target_bir_lowering=False, debug=False, num_devices=N_CORES)

    obs8 = nc.dram_tensor("obs8", [NP, 128, 2, NG, 2, TT], f8, kind="ExternalInput").ap()
    w1p = nc.dram_tensor("w1p", [128, NG, 2, H], f8, kind="ExternalInput").ap()
    w2p = nc.dram_tensor("w2p", [128, NG, 2, H], f8, kind="ExternalInput").ap()
    # w3 | b1 | b2 | b3 | rmat packed in one tensor -> one small DMA that
    # goes first, so the bias needed by the first activation isn't queued
    # behind megabytes of weight/obs traffic
    consts = nc.dram_tensor("consts", [128, 25 + RCOLS], f32, kind="ExternalInput").ap()
    rmatb = nc.dram_tensor("rmatb", [128, RCOLS], bf16, kind="ExternalInput").ap()
    scanm = nc.dram_tensor("scanm", [128, NJ + 1, 128], bf16, kind="ExternalInput").ap()
    # output stays in [t%128, t/128] tile layout; the host un-permutes.
    # (a "(b p) -> p b" DRAM view would scatter 4-byte words, whose DMA
    # completion costs ~14us at the end of the program)
    outv = nc.dram_tensor("out", [128, NB], f32, kind="ExternalOutput").ap()
    # V in row layout, one [1, TT] row per timestep tile; the host does
    # the final out = G - b3 - V combine during un-permutation
    vout = nc.dram_tensor("vout", [2 * NP, 1, TT], f32, kind="ExternalOutput").ap()

    with tile.TileContext(nc) as tc, ExitStack() as ctx:
        const = ctx.enter_context(tc.tile_pool(name="const", bufs=1))
        w1_sb = const.tile([128, NG, 2, H], f8, name="w1_sb")
        w2_sb = const.tile([128, NG, 2, H], f8, name="w2_sb")
        scan_sb = const.tile([128, NJ + 1, 128], bf16, name="scan_sb")
        rmat_sb = const.tile([128, RCOLS], bf16, name="rmat_sb")
        cst = const.tile([128, 25 + RCOLS], f32, name="cst")
        w3_sb = cst[:, 0:8]
        b1_sb = cst[:, 8:16]
        b2_sb = cst[:, 16:24]
        b3_sb = cst[:, 24:25]
        identb = const.tile([128, 128], bf16, name="identb")

        otp = ctx.enter_context(tc.tile_pool(name="otp", bufs=3))
        h1p = ctx.enter_context(tc.tile_pool(name="h1p", bufs=2))
        h2p = ctx.enter_context(tc.tile_pool(name="h2p", bufs=4))
        accp = ctx.enter_context(tc.tile_pool(name="accp", bufs=4))
        vrp = ctx.enter_context(tc.tile_pool(name="vrp", bufs=3))
        mmp = ctx.enter_context(tc.tile_pool(name="mmp", bufs=7, space="PSUM"))

        gsbp = ctx.enter_context(tc.tile_pool(name="gsbp", bufs=1))
        g_sb = gsbp.tile([128, NB], f32, name="g_sb")

        # ---- startup DMAs, ordered so PE can start within a few us ----
        # consts (biases!) first, then pair-0 obs j=0 interleaved with W1
        # by contraction group -- pair 0's layer-1 matmuls are issued
        # j-major so the first matmul only needs the (j=0, g=0) chunks.
        ots = {}
        ots[0] = otp.tile([128, 2, NG, 2, TT], f8, tag="ot", name="ot_0")
        nc.sync.dma_start(ots[0][:, 0, 0], obs8[0][:, 0, 0])
        nc.sync.dma_start(cst[:], consts[:])
        nc.sync.dma_start(w1_sb[:, 0, :, 0:512], w1p[:, 0, :, 0:512])
        nc.sync.dma_start(rmat_sb[:], rmatb[:])
        nc.sync.dma_start(w1_sb[:, 0, :, 512:H], w1p[:, 0, :, 512:H])
        for g in range(1, NG):
            nc.sync.dma_start(ots[0][:, 0, g], obs8[0][:, 0, g])
            nc.sync.dma_start(w1_sb[:, g, :, :], w1p[:, g, :, :])
        nc.sync.dma_start(ots[0][:, 1], obs8[0][:, 1])
        ots[1] = otp.tile([128, 2, NG, 2, TT], f8, tag="ot", name="ot_1")
        nc.sync.dma_start(ots[1][:], obs8[1])
        nc.sync.dma_start(w2_sb[:], w2p[:])
        nc.sync.dma_start(scan_sb[:], scanm[:])
        make_identity(nc, identb[:])
        ones_sb = const.tile([128, 1], bf16, name="ones_sb")
        nc.vector.memset(ones_sb[:], 1.0)
        # warm the scalar engine's activation table during the DMA window
        # (the first ACTIVATE otherwise pays a ~1.3us table load, which
        # stalls PSUM-bank recycling mid-stream)
        actw = const.tile([128, 1], f32, name="actw")
        nc.scalar.activation(actw[:], identb[:, 0:1], Relu)
        # warm the PE HAM clock gate during the DMA-wait head: ~3.5us of
        # dummy transposes releases the 1.2 GHz cold throttle before the
        # real matmul stream begins
        warm_ps = mmp.tile([128, 128], bf16, tag="mm", name="warm_ps")
        for _ in range(32):
            nc.tensor.transpose(warm_ps[:], identb[:], identb[:])

        for pk in range(NP):
            if pk + 2 < NP:
                ot = otp.tile([128, 2, NG, 2, TT], f8, tag="ot", name=f"ot_{pk + 2}")
                nc.sync.dma_start(ot[:], obs8[pk + 2])
                ots[pk + 2] = ot

            # ---- layer 1: h1 = relu(obs @ W1 + b1), fp8 DoubleRow ----
            # pair 0 runs j-major so the first matmuls only depend on the
            # j=0 half of the obs DMA; later pairs interleave j so a
            # stationary weight load serves two 512-column streams.
            h1pair = h1p.tile([128, 2, NG, 2, TT], f8, tag="h1", name=f"h1_{pk}")
            if pk == 0:
                for j in range(2):
                    for ho in range(8):
                        p1 = mmp.tile([128, TT], f32, tag="mm", name=f"p1_0_{ho}_{j}")
                        for g in range(NG):
                            nc.tensor.matmul(
                                p1[:],
                                lhsT=w1_sb[:, g, :, ho * 128 : (ho + 1) * 128],
                                rhs=ots[0][:, j, g, :, :],
                                start=(g == 0),
                                stop=(g == NG - 1),
                                perf_mode=DR,
                            )
                        nc.scalar.activation(
                            h1pair[:, j, ho // 2, ho % 2, :],
                            p1[:],
                            Relu,
                            bias=b1_sb[:, ho : ho + 1],
                            scale=1.0 / WSCALE,
                        )
            else:
                for ho in range(8):
                    p1 = [
                        mmp.tile([128, TT], f32, tag="mm", name=f"p1_{pk}_{ho}_{j}")
                        for j in range(2)
                    ]
                    for g in range(NG):
                        for j in range(2):
                            nc.tensor.matmul(
                                p1[j][:],
                                lhsT=w1_sb[:, g, :, ho * 128 : (ho + 1) * 128],
                                rhs=ots[pk][:, j, g, :, :],
                                start=(g == 0),
                                stop=(g == NG - 1),
                                perf_mode=DR,
                            )
                    for j in range(2):
                        nc.scalar.activation(
                            h1pair[:, j, ho // 2, ho % 2, :],
                            p1[j][:],
                            Relu,
                            bias=b1_sb[:, ho : ho + 1],
                            scale=1.0 / WSCALE,
                        )

            # ---- layer 2 + V lane accumulation (bf16 transposes) ----
            accs = [
                accp.tile([128, TT], bf16, tag="acc", name=f"acc_{pk}_{j}")
                for j in range(2)
            ]
            for ho in range(8):
                p2 = [
                    mmp.tile([128, TT], f32, tag="mm", name=f"p2_{pk}_{ho}_{j}")
                    for j in range(2)
                ]
                for g in range(NG):
                    for j in range(2):
                        nc.tensor.matmul(
                            p2[j][:],
                            lhsT=w2_sb[:, g, :, ho * 128 : (ho + 1) * 128],
                            rhs=h1pair[:, j, g, :, :],
                            start=(g == 0),
                            stop=(g == NG - 1),
                            perf_mode=DR,
                        )
                for j in range(2):
                    h2 = h2p.tile([128, TT], bf16, tag="h2", name=f"h2_{pk}_{ho}_{j}")
                    nc.scalar.activation(
                        h2[:],
                        p2[j][:],
                        Relu,
                        bias=b2_sb[:, ho : ho + 1],
                        scale=1.0 / WSCALE,
                    )
                    if ho == 0:
                        nc.vector.tensor_scalar_mul(accs[j][:], h2[:], w3_sb[:, 0:1])
                    else:
                        nc.vector.scalar_tensor_tensor(
                            accs[j][:],
                            h2[:],
                            w3_sb[:, ho : ho + 1],
                            accs[j][:],
                            AluOpType.mult,
                            AluOpType.add,
                        )
                    if ho == 7:
                        it = 2 * pk + j
                        if pk < NP - 1:
                            # V = sum over the 128 lanes on the idle GpSimd
                            # engine (partition all-reduce ucode, ~3.8us,
                            # fully hidden mid-stream); DMA one row out
                            vrow = vrp.tile([128, TT], f32, tag="vr", name=f"vr_{it}")
                            nc.gpsimd.partition_all_reduce(
                                vrow[:], accs[j][:], 128, bass_isa.ReduceOp.add
                            )
                            nc.sync.dma_start(vout[it], vrow[0:1, :])
                        else:
                            # last pair: the GpSimd ucode would serialize
                            # ~8us after the matmul stream ends; a
                            # ones-vector matmul gets V in 216ns instead
                            vps = mmp.tile([1, TT], f32, tag="mm", name=f"vps_{it}")
                            nc.tensor.matmul(
                                vps[:], lhsT=ones_sb[:], rhs=accs[j][:].bitcast(bf16)
                            )
                            vrow = vrp.tile([1, TT], f32, tag="vr", name=f"vr_{it}")
                            nc.vector.tensor_copy(vrow[:], vps[:])
                            nc.sync.dma_start(vout[it], vrow[:])

            if pk == 0:
                # discounted returns: 17 banded matmuls, fp32
                g_psum = mmp.tile([128, NB], f32, tag="mm", name="g_psum")
                for j in range(NJ + 1):
                    nc.tensor.matmul(
                        g_psum[:],
                        lhsT=scan_sb[:, j, :],
                        rhs=rmat_sb[:, j : j + NB],
                        start=(j == 0),
                        stop=(j == NJ),
                    )
                nc.scalar.activation(g_sb[:], g_psum[:], Copy)
                nc.sync.dma_start(outv[:], g_sb[:])


    nc.compile()
    _cache["nc"] = nc
    return nc


def _pack_inputs(rewards, obs, W1, b1, W2, b2, W3, b3):
    import ml_dtypes

    f8 = ml_dtypes.float8_e4m3
    bf16 = ml_dtypes.bfloat16

    scanm = np.ascontiguousarray(_scan_mats().transpose(1, 0, 2).astype(bf16))
    w3p = W3.reshape(8, 128).T
    b1p = b1.reshape(8, 128).T
    b2p = b2.reshape(8, 128).T
    b3c = np.broadcast_to(b3.reshape(1, 1), (128, 1))

    # weights: pre-scale into fp8 normal range, pack d = 256g + 128i + p
    w1q = np.ascontiguousarray(
        (W1 * WSCALE).astype(f8).reshape(NG, 2, 128, H).transpose(2, 0, 1, 3)
    )
    w2q = np.ascontiguousarray(
        (W2 * WSCALE).astype(f8).reshape(NG, 2, 128, H).transpose(2, 0, 1, 3)
    )

    obs_q = obs.astype(f8)  # [T, D] fp8, cast once

    r_pad = np.zeros(T + WIN, dtype=np.float32)
    r_pad[:T] = rewards

    in_maps = []
    for c in range(N_CORES):
        lo = c * TC
        o8 = np.ascontiguousarray(
            obs_q[lo : lo + TC]
            .reshape(NP, 2, TT, NG, 2, 128)
            .transpose(0, 5, 1, 3, 4, 2)
        )
        rmat = r_pad[lo : lo + TC + WIN].reshape(RCOLS, 128).T
        cst = np.concatenate([w3p, b1p, b2p, b3c, rmat], axis=1)
        in_maps.append(
            {
                "obs8": o8,
                "w1p": w1q,
                "w2p": w2q,
                "consts": np.ascontiguousarray(cst, dtype=np.float32),
                "rmatb": np.ascontiguousarray(rmat.astype(bf16)),
                "scanm": scanm,
            }
        )
    return in_maps


def kernel(rewards, obs, W1, b1, W2, b2, W3, b3):
    from concourse.bass_utils import run_bass_kernel_spmd

    rewards = np.asarray(rewards, dtype=np.float32)
    obs = np.asarray(obs, dtype=np.float32)
    W1 = np.ascontiguousarray(np.asarray(W1, dtype=np.float32))
    W2 = np.ascontiguousarray(np.asarray(W2, dtype=np.float32))
    W3 = np.asarray(W3, dtype=np.float32)
    b1 = np.asarray(b1, dtype=np.float32)
    b2 = np.asarray(b2, dtype=np.float32)
    b3 = np.asarray(b3, dtype=np.float32)

    nc = _build()
    in_maps = _pack_inputs(rewards, obs, W1, b1, W2, b2, W3, b3)
    res = run_bass_kernel_spmd(nc, in_maps, core_ids=list(range(N_CORES)))
    # device "out" is G in [t%128, t/128] layout, "vout" is V in row
    # layout; un-permute G and fold in b3 and V on the host
    parts = []
    for c in range(N_CORES):
        g = np.ascontiguousarray(res.results[c]["out"].T).reshape(TC)
        v = res.results[c]["vout"].reshape(TC)
        parts.append(g - (v + b3[0]))
    return np.concatenate(parts)
